# revision 9
# baseline (speedup 1.0000x reference)
"""Distributed GCN (2x GCNConv + MLP head) on 8 Trainium2 NeuronCores — v2.

Pipeline redesign vs v1:
  - Chunked AllGather (P_AG sub-collectives per layer over shard row-slices);
    message gathers are gated per-chunk on the sub-AG that delivers their
    sources, so the exchange overlaps the gather/compute stream.
  - Exchange table in fp8e4m3 (rows pre-scaled by dinv[src]; dinv[dst] folded
    into the bf16 one-hot S matrix). AllGather runs on a bf16 bitcast view.
  - Self-loop contributions use a local bf16 scaled shard (no fp8 error).
  - Edges spill-packed into 128-slot chunks sorted by (source-slice, row);
    chunk schedule is group-major (all chunks needing sub-AG g dispatch
    together) with per-(block,group) PSUM sessions; cross-session carry via
    ACT copy to SBUF + identity-matmul re-injection.
  - S one-hot build in bf16 (2x DVE rate); psum evacs moved to ACT.
"""

import contextlib

import numpy as np
import ml_dtypes

import concourse.bass as bass
import concourse.mybir as mybir
from concourse.bass_utils import run_bass_kernel_spmd

FP32 = mybir.dt.float32
BF16 = mybir.dt.bfloat16
FP8 = mybir.dt.float8e4
I32 = mybir.dt.int32
AF = mybir.ActivationFunctionType
ALU = mybir.AluOpType

C = 8
PB = 128
import os
TABLE_FP8 = os.environ.get("TABLE_FP8", "1") == "1"
PARTIAL_ACT = os.environ.get("PARTIAL_ACT", "1") == "1"
GQN = int(os.environ.get("GQN", "4"))
TDT = FP8 if TABLE_FP8 else BF16
NRG = 48   # fp8 msg ring slots (chunks)
NSR = 8    # self-msg ring slots (bf16)
NS = 48    # S ring slots
ACHUNK = 512

bfd = ml_dtypes.bfloat16
f8d = ml_dtypes.float8_e4m3


class Cfg:
    def __init__(self, n, e, d_in, h, d_out, nb, p_ag):
        self.N = n
        self.E = e
        self.DIN = d_in
        self.H = h
        self.DOUT = d_out
        self.NB = nb
        self.NLOC = nb * PB
        self.P_AG = p_ag
        # block split for sub-AGs: nearly equal, first slices take remainder
        base, rem = nb // p_ag, nb % p_ag
        self.BS = [0]
        for p in range(p_ag):
            self.BS.append(self.BS[-1] + base + (1 if p < rem else 0))
        self.RS = [b * PB for b in self.BS]
        assert self.NLOC * C >= n


FULL = Cfg(50000, 300000, 128, 256, 256, 49, int(os.environ.get("P_AG", "2")))


# ---------------------------------------------------------------- host preproc
def preprocess(cfg, x, edge_index):
    N, NB, P_AG = cfg.N, cfg.NB, cfg.P_AG
    nblocks = C * NB
    src = np.asarray(edge_index[0], dtype=np.int64)
    dst = np.asarray(edge_index[1], dtype=np.int64)
    indeg = np.bincount(dst, minlength=N).astype(np.int64)
    deg = indeg + 1
    dinv = (1.0 / np.sqrt(deg.astype(np.float64))).astype(np.float32)

    # balanced assignment of nodes to blocks (weight = deg incl self-loop)
    w = deg
    order = np.argsort(-w, kind="stable")
    import heapq

    heap = [(0, b) for b in range(nblocks)]
    heapq.heapify(heap)
    cnt = np.zeros(nblocks, np.int64)
    sumw = np.zeros(nblocks, np.int64)
    blk_of = np.empty(N, np.int32)
    slot_of = np.empty(N, np.int32)
    for v in order:
        while True:
            sw, b = heapq.heappop(heap)
            if cnt[b] < PB:
                break
        blk_of[v] = b
        slot_of[v] = cnt[b]
        cnt[b] += 1
        sumw[b] += w[v]
        if cnt[b] < PB:
            heapq.heappush(heap, (sumw[b], b))

    core_of_node = (blk_of // NB).astype(np.int32)
    lblk_of_node = (blk_of % NB).astype(np.int32)
    loc_of_node = lblk_of_node.astype(np.int64) * PB + slot_of
    new_row = core_of_node.astype(np.int64) * cfg.NLOC + loc_of_node

    # table row in sliced layout: slice p holds rows RS[p]..RS[p+1] of every
    # core, concatenated core-major within the slice.
    RS = np.asarray(cfg.RS, np.int64)
    slice_of_loc = np.searchsorted(RS, np.arange(cfg.NLOC), side="right") - 1

    def t_row(core, loc):
        p = slice_of_loc[loc]
        return C * RS[p] + core * (RS[p + 1] - RS[p]) + (loc - RS[p])

    e_srow = t_row(core_of_node[src], loc_of_node[src])
    e_slice = slice_of_loc[loc_of_node[src]]
    e_blk = blk_of[dst]
    e_dslot = slot_of[dst]
    e_norm = dinv[dst]  # dinv[src] folded into the table rows

    # per (core, lb): sorted edge lists
    sort_key = e_blk.astype(np.int64) * (P_AG * cfg.NLOC * C + 1) * 2
    sort_key += e_slice.astype(np.int64) * (cfg.NLOC * C + 1) + e_srow
    so = np.argsort(sort_key, kind="stable")
    e_srow, e_slice = e_srow[so], e_slice[so]
    e_blk, e_dslot, e_norm = e_blk[so], e_dslot[so], e_norm[so]
    starts = np.searchsorted(e_blk, np.arange(nblocks))
    ends = np.searchsorted(e_blk, np.arange(nblocks) + 1)

    # chunks per block position lb: max over cores
    MB = np.zeros(NB, np.int64)
    for b in range(nblocks):
        lb = b % NB
        n = ends[b] - starts[b]
        MB[lb] = max(MB[lb], -(-n // PB))
    # per-chunk group: max over cores of slice of last real edge in chunk
    G = [np.zeros(MB[lb], np.int64) for lb in range(NB)]
    for b in range(nblocks):
        lb = b % NB
        s, e = starts[b], ends[b]
        n = e - s
        for i in range(int(MB[lb])):
            last = min(n, (i + 1) * PB) - 1
            if last >= i * PB:
                G[lb][i] = max(G[lb][i], e_slice[s + last])
    for lb in range(NB):  # monotone (should already be)
        np.maximum.accumulate(G[lb], out=G[lb])

    # dispatch schedule: group-major; self chunk first in group 0
    CH = []  # (lb, i) with i == -1 for self chunk
    for g in range(P_AG):
        for lb in range(NB):
            if g == 0:
                CH.append((lb, -1))
            for i in range(int(MB[lb])):
                if G[lb][i] == g:
                    CH.append((lb, i))
    TOTCH = len(CH)
    ch_group = [0] * TOTCH
    for d, (lb, i) in enumerate(CH):
        ch_group[d] = 0 if i == -1 else int(G[lb][i])

    # sessions: per lb ordered list of (g, [dispatch indices])
    SES = {lb: [] for lb in range(NB)}
    for d, (lb, i) in enumerate(CH):
        g = ch_group[d]
        if SES[lb] and SES[lb][-1][0] == g:
            SES[lb][-1][1].append(d)
        else:
            SES[lb].append((g, [d]))

    # fill per-core chunk data arrays
    esrcT = np.zeros((C, PB, TOTCH), np.int32)
    edstT = np.zeros((C, PB, TOTCH), np.float32)
    enormT = np.zeros((C, PB, TOTCH), np.float32)
    dinv_col = np.zeros((C, PB, NB), np.float32)
    for c in range(C):
        nodes = np.where(core_of_node == c)[0]
        dinv_col[c, slot_of[nodes], lblk_of_node[nodes]] = dinv[nodes]
    for d, (lb, i) in enumerate(CH):
        if i == -1:
            # self chunk: local shard rows lb*PB + slot, norm dinv, dslot slot
            for c in range(C):
                esrcT[c, :, d] = lb * PB + np.arange(PB)
                edstT[c, :, d] = np.arange(PB)
                enormT[c, :, d] = dinv_col[c, :, lb]
        else:
            for c in range(C):
                b = c * NB + lb
                s, e = starts[b], ends[b]
                lo, hi = s + i * PB, min(e, s + (i + 1) * PB)
                nn = max(0, hi - lo)
                if nn > 0:
                    esrcT[c, :nn, d] = e_srow[lo:hi]
                    edstT[c, :nn, d] = e_dslot[lo:hi]
                    enormT[c, :nn, d] = e_norm[lo:hi]
                # pad rows: esrc 0 (row 0, slice 0 → always arrived), norm 0

    xT = np.zeros((C, cfg.DIN, cfg.NLOC), bfd)
    xx = np.asarray(x, np.float32)
    for c in range(C):
        m = core_of_node == c
        xT[c][:, loc_of_node[m]] = xx[m].T.astype(bfd)

    return dict(
        KC=int(MB.max()), MB=MB, G=G, CH=CH, SES=SES, TOTCH=TOTCH,
        ch_group=ch_group,
        esrcT=esrcT, edstT=edstT, enormT=enormT,
        dinv_col=dinv_col, xT=xT, new_row=new_row,
    )


def pack_weights(cfg, W1, b1, Wg1, bg1, Wg2, bg2, W2, b2, W3, b3):
    def packk(Wm):
        k = Wm.shape[0] // 128
        return np.concatenate([Wm[i * 128 : (i + 1) * 128] for i in range(k)], axis=1)

    def bias2(bv):
        return np.stack([bv[:128], bv[128:]], axis=1).astype(np.float32)

    return dict(
        W1=np.asarray(W1, np.float32).astype(bfd),
        b1p=bias2(np.asarray(b1)),
        wg1p=packk(np.asarray(Wg1)).astype(bfd),
        bg1p=bias2(np.asarray(bg1)),
        wg2p=packk(np.asarray(Wg2)).astype(bfd),
        bg2p=bias2(np.asarray(bg2)),
        w2p=packk(np.asarray(W2)).astype(bfd),
        b2p=bias2(np.asarray(b2)),
        w3p=packk(np.asarray(W3)).astype(bfd),
        b3bc=np.tile(np.asarray(b3, np.float32)[None, :], (PB, 1)),
        iota=np.tile(np.arange(PB, dtype=np.float32)[None, :], (PB, 1)),
        ident=np.eye(PB, dtype=np.float32).astype(bfd),
    )


# ------------------------------------------------------------- op-list program
class Prog:
    ENGS = ("sync", "pe", "dve", "act", "pool")

    def __init__(self):
        self.ops = {e: [] for e in self.ENGS}
        self.tick = {}

    def emit(self, eng, fn, waits=(), inc=None, inc_by=1):
        t = None
        if inc is not None:
            self.tick[inc] = self.tick.get(inc, 0) + inc_by
            t = self.tick[inc]
        self.ops[eng].append((fn, tuple(waits), inc, inc_by))
        return t


def fold(waits):
    m = {}
    for s, v in waits:
        if v is None:
            continue
        m[s] = max(m.get(s, 0), v)
    return list(m.items())


def build_graph(cfg, prep, with_cc=True, with_gather=True):
    NB, NLOC, H, DOUT, P_AG = cfg.NB, cfg.NLOC, cfg.H, cfg.DOUT, cfg.P_AG
    CH, SES, TOTCH = prep["CH"], prep["SES"], prep["TOTCH"]
    ch_group = prep["ch_group"]
    NCH_A = -(-NLOC // ACHUNK)

    nc = bass.Bass(num_swdge_queues=max(GQN, 1))

    def dparam(name, shape, dt):
        return nc.declare_dram_parameter(name, shape, dt, isOutput=False)

    p_xT = dparam("xT", [cfg.DIN, NLOC], BF16)
    p_W1 = dparam("W1", [cfg.DIN, H], BF16)
    p_b1p = dparam("b1p", [PB, 2], FP32)
    p_wg1p = dparam("wg1p", [PB, 2 * H], BF16)
    p_bg1p = dparam("bg1p", [PB, 2], FP32)
    p_wg2p = dparam("wg2p", [PB, 2 * H], BF16)
    p_bg2p = dparam("bg2p", [PB, 2], FP32)
    p_w2p = dparam("w2p", [PB, 2 * H], BF16)
    p_b2p = dparam("b2p", [PB, 2], FP32)
    p_w3p = dparam("w3p", [PB, 2 * DOUT], BF16)
    p_b3bc = dparam("b3bc", [PB, DOUT], FP32)
    p_iota = dparam("iota", [PB, PB], FP32)
    p_ident = dparam("ident", [PB, PB], BF16)
    p_esrc = dparam("esrcT", [PB, TOTCH], I32)
    p_edst = dparam("edstT", [PB, TOTCH], FP32)
    p_enorm = dparam("enormT", [PB, TOTCH], FP32)
    p_dinv = dparam("dinv_col", [PB, NB], FP32)
    p_out = nc.declare_dram_parameter("out", [NLOC, DOUT], FP32, isOutput=True)

    # exchange tables (fp8, AG'd) + local bf16 scaled shards for self loops
    f8_shard = [nc.dram_tensor(f"f8s{i}", [NLOC, H], TDT) for i in (1, 2)]
    f8_full = [
        nc.dram_tensor(f"f8f{i}", [C * NLOC, H], TDT, addr_space="Shared")
        for i in (1, 2)
    ]

    with contextlib.ExitStack() as ctx:
        sb = lambda name, shape, dt: ctx.enter_context(nc.sbuf_tensor(name, shape, dt))
        pst = lambda name, shape: ctx.enter_context(nc.psum_tensor(name, shape, FP32))

        xT_sb = sb("xT_sb", [cfg.DIN, NLOC], BF16)
        W1_sb = sb("W1_sb", [cfg.DIN, H], BF16)
        b1p_sb = sb("b1p_sb", [PB, 2], FP32)
        wg1_sb = sb("wg1_sb", [PB, 2 * H], BF16)
        bg1p_sb = sb("bg1p_sb", [PB, 2], FP32)
        wg2_sb = sb("wg2_sb", [PB, 2 * H], BF16)
        bg2p_sb = sb("bg2p_sb", [PB, 2], FP32)
        w2_sb = sb("w2_sb", [PB, 2 * H], BF16)
        b2p_sb = sb("b2p_sb", [PB, 2], FP32)
        w3_sb = sb("w3_sb", [PB, 2 * DOUT], BF16)
        b3bc_sb = sb("b3bc_sb", [PB, DOUT], FP32)
        iota_sb = sb("iota_sb", [PB, PB], FP32)
        id_sb = sb("id_sb", [PB, PB], BF16)
        esrc_sb = sb("esrc_sb", [PB, TOTCH], I32)
        edst_sb = sb("edst_sb", [PB, TOTCH], FP32)
        enorm_sb = sb("enorm_sb", [PB, TOTCH], FP32)
        dinv_sb = sb("dinv_sb", [PB, NB], FP32)

        # h1/h3 aliased, h2/h4 aliased
        hA = [sb(f"hA_{j}", [PB, NLOC], BF16) for j in range(2)]
        hB = [sb(f"hB_{j}", [PB, NLOC], BF16) for j in range(2)]
        hT = {1: hA, 2: hB, 3: hA, 4: hB}
        pb_sb = sb("pb_sb", [PB, NB * 2 * PB], BF16)
        msg_ring = sb("msg_ring", [PB, NRG * H], TDT)
        s_ring = sb("s_ring", [PB, NS * PB], BF16)
        hwNM = sb("hwNM", [PB, NB * H], BF16)
        fev_ring = sb("fev_ring", [PB, 4 * H], TDT)
        lg_ring = sb("lg_ring", [PB, 2 * DOUT], FP32)
        ex_ring = sb("ex_ring", [PB, 2 * DOUT], FP32)
        ot_ring = sb("ot_ring", [PB, 2 * DOUT], FP32)
        sm_cols = sb("sm_cols", [PB, 8], FP32)

        psA = [pst(f"psA_{i}", [PB, ACHUNK]) for i in range(2)]
        psB = [pst(f"psB_{i}", [PB, H]) for i in range(2)]
        psD = [pst(f"psD_{i}", [PB, 2 * PB]) for i in range(4)]

        P = Prog()

        # ------------- const loads
        loads = [
            (xT_sb, p_xT), (W1_sb, p_W1), (b1p_sb, p_b1p), (wg1_sb, p_wg1p),
            (bg1p_sb, p_bg1p), (wg2_sb, p_wg2p), (bg2p_sb, p_bg2p),
            (w2_sb, p_w2p), (b2p_sb, p_b2p), (w3_sb, p_w3p), (b3bc_sb, p_b3bc),
            (iota_sb, p_iota), (id_sb, p_ident), (esrc_sb, p_esrc),
            (edst_sb, p_edst), (enorm_sb, p_enorm), (dinv_sb, p_dinv),
        ]
        for t, pp in loads:
            P.emit("sync", (lambda t=t, pp=pp: lambda e: e.dma_start(t[:], pp[:]))(),
                   inc="c16", inc_by=16)
        C16_ALL = P.tick["c16"]
        for eng in ("pe", "dve", "act", "pool"):
            P.emit(eng, lambda e: None, waits=[("c16", C16_ALL)])

        psA_hist = []
        psB_hist = []
        psD_hist = [None] * 4   # bank -> [(sem, tick)] of last evacs
        bev_hist = []           # (sem, tick) of shard-write DMA per bev alloc
        fev_hist = []
        ot_hist = []
        bev_t = {}              # (layer, lb) -> dve tick of hwNM evac
        fw_write = {}           # (layer, lb) -> ("fw{slot}", tick)
        mm_of_chunk = {}        # (layer, d) -> pe tick of 2nd matmul
        gath_t = {}             # (layer, d) -> (sem, tick) of gather
        sbuild_t = {}           # (layer, d) -> dve tick of S build
        evF = {}                # (layer, lb) -> act tick of final evac
        cc_t = {}               # (layer, g) -> cc1 tick

        # ------------- stage A + B1 interleaved; sub-AG1 triggered en route
        a_evt = {}

        def emit_A_chunk(si):
            s = si * ACHUNK
            wd = min(ACHUNK, NLOC - s)
            for j in range(2):
                ai = len(psA_hist)
                ps = psA[ai % 2]
                waits = [psA_hist[ai - 2]] if ai >= 2 else []
                P.emit("pe",
                       (lambda ps=ps, j=j, s=s, wd=wd: lambda e: e.matmul(
                           ps[:, :wd], lhsT=W1_sb[:, j * PB : (j + 1) * PB],
                           rhs=xT_sb[:, s : s + wd], start=True, stop=True))(),
                       waits=fold(waits), inc="pe1")
                mmt = P.tick["pe1"]
                t = P.emit("act",
                           (lambda ps=ps, j=j, s=s, wd=wd: lambda e: e.activation(
                               hT[1][j][:, s : s + wd], ps[:, :wd], AF.Relu,
                               bias=b1p_sb[:, j : j + 1]))(),
                           waits=[("pe1", mmt)], inc="act1")
                a_evt[(j, si)] = t
                psA_hist.append(("act1", t))

        def emit_B_block(li, hin_key, wg_sb, lb, ready):
            """hW matmul for block lb + fp8/bf16 evacs + shard writes."""
            bi = len(psB_hist)
            ps = psB[bi % 2]
            hin = hT[hin_key]
            waits = [ready]
            if bi >= 2:
                waits.extend(psB_hist[bi - 2])
            P.emit("pe",
                   (lambda ps=ps, hin=hin, lb=lb, wg_sb=wg_sb: lambda e: e.matmul(
                       ps[:], lhsT=hin[0][:, lb * PB : (lb + 1) * PB],
                       rhs=wg_sb[:, 0:H], start=True, stop=False))(),
                   waits=fold(waits), inc="pe1")
            P.emit("pe",
                   (lambda ps=ps, hin=hin, lb=lb, wg_sb=wg_sb: lambda e: e.matmul(
                       ps[:], lhsT=hin[1][:, lb * PB : (lb + 1) * PB],
                       rhs=wg_sb[:, H : 2 * H], start=False, stop=True))(),
                   inc="pe1")
            mmt = P.tick["pe1"]
            slot = len(fev_hist) % 4
            # bf16 scaled evac (node-major, persistent for the self matmul).
            # WAR vs layer-1 self matmul is implicit: this waits our own B
            # matmul tick, which is far beyond L1's self-chunk matmuls.
            t_bev = P.emit("dve",
                           (lambda ps=ps, lb=lb: lambda e:
                            e.tensor_scalar_mul(
                                hwNM[:, lb * H : (lb + 1) * H], ps[:],
                                dinv_sb[:, lb : lb + 1]))(),
                           waits=[("pe1", mmt)], inc="dve1")
            bev_t[(li, lb)] = t_bev
            # fp8 scaled evac on DVE
            fw_ = [("pe1", mmt)]
            if len(fev_hist) >= 4:
                fw_.append(fev_hist[len(fev_hist) - 4])
            t_fev = P.emit("dve",
                           (lambda ps=ps, slot=slot, lb=lb: lambda e:
                            e.tensor_scalar_mul(
                                fev_ring[:, slot * H : (slot + 1) * H], ps[:],
                                dinv_sb[:, lb : lb + 1]))(),
                           waits=fold(fw_), inc="dve1")
            psB_hist.append([("dve1", t_fev), ("dve1", t_bev)])
            P.emit("sync",
                   (lambda li=li, lb=lb, slot=slot: lambda e: e.dma_start(
                       f8_shard[li - 1][lb * PB : (lb + 1) * PB, :],
                       fev_ring[:, slot * H : (slot + 1) * H]))(),
                   waits=[("dve1", t_fev)], inc=f"fw{slot}", inc_by=16)
            fw_write[(li, lb)] = (f"fw{slot}", P.tick[f"fw{slot}"])
            fev_hist.append(fw_write[(li, lb)])

        def emit_subAG(li, g):
            if not with_cc:
                cc_t[(li, g)] = 0
                return
            r0, r1 = cfg.RS[g], cfg.RS[g + 1]
            waits = [fw_write[(li, lb)] for lb in range(cfg.BS[g], cfg.BS[g + 1])]
            shard_ap = f8_shard[li - 1][r0:r1, :]
            full_ap = f8_full[li - 1][C * r0 : C * r1, :]
            if TABLE_FP8:
                shard_ap = shard_ap.bitcast(BF16)
                full_ap = full_ap.bitcast(BF16)
            P.emit("pool",
                   (lambda shard_ap=shard_ap, full_ap=full_ap: lambda e:
                    e.collective_compute(
                        "AllGather", ALU.bypass,
                        replica_groups=[list(range(C))],
                        ins=[shard_ap.opt()], outs=[full_ap.opt()]))(),
                   waits=fold(waits), inc="cc1")
            cc_t[(li, g)] = P.tick["cc1"]

        # ------------- per-layer message passing
        reg_ds = [d for d, (lb_, i_) in enumerate(CH) if i_ != -1]
        self_ds = [d for d, (lb_, i_) in enumerate(CH) if i_ == -1]
        sess_ctr = [0]

        def emit_layer(li, hout_key, bgp_sb, after_final):
            """after_final(lb): emit post-completion work for block lb."""
            nreg = 0  # regular chunk counter (msg ring)
            nself = 0
            ses_idx = {lb: 0 for lb in range(NB)}
            chunk_loc = {}

            for d, (lb, i) in enumerate(CH):
                g = ch_group[d]
                # --- gather (pool)
                if i == -1:
                    gath_t[(li, d)] = ("dve1", bev_t[(li, lb)])
                    nself += 1
                    lhs_ring, lhs_off = hwNM, lb * H
                else:
                    r = nreg % NRG
                    gw = []
                    if with_cc:
                        gw.append(("cc1", cc_t[(li, g)]))
                    if nreg >= NRG and nreg % 8 == 0:
                        lim = min(nreg - NRG + 7, len(reg_ds) - 1)
                        gw.append(("pe1", mm_of_chunk[(li, reg_ds[lim])]))
                    if with_gather:
                        P.emit("pool",
                               (lambda li=li, d=d, r=r: lambda e:
                                e.indirect_dma_start(
                                    out=msg_ring[:, r * H : (r + 1) * H],
                                    out_offset=None,
                                    in_=f8_full[li - 1][:, :],
                                    in_offset=bass.IndirectOffsetOnAxis(
                                        ap=esrc_sb[:, d : d + 1], axis=0)))(),
                               waits=fold(gw), inc=f"gs{r}", inc_by=16)
                        gath_t[(li, d)] = (f"gs{r}", P.tick[f"gs{r}"])
                    else:
                        gath_t[(li, d)] = None
                    nreg += 1
                    lhs_ring, lhs_off = msg_ring, r * H

                # --- S build (dve); ring WAR coarsened to 1-in-8
                sr = d % NS
                sw = []
                if d >= NS and d % 8 == 0:
                    lim = min(d - NS + 7, TOTCH - 1)
                    sw.append(("pe1", mm_of_chunk[(li, lim)]))
                sbuild_t[(li, d)] = P.emit(
                    "dve",
                    (lambda d=d, sr=sr: lambda e: e.tensor_scalar(
                        out=s_ring[:, sr * PB : (sr + 1) * PB], in0=iota_sb[:],
                        scalar1=edst_sb[:, d : d + 1],
                        scalar2=enorm_sb[:, d : d + 1],
                        op0=ALU.is_equal, op1=ALU.mult))(),
                    waits=fold(sw), inc="dve1")

                # --- matmuls (pe): batched per session at its last chunk
                sess_list = SES[lb]
                si = ses_idx[lb]
                cur_g, members = sess_list[si]
                last_of_sess = d == members[-1]
                last_sess = si == len(sess_list) - 1
                chunk_loc[d] = (lhs_ring, lhs_off, sr)
                if not last_of_sess:
                    continue

                bank = sess_ctr[0] % 4
                sess_ctr[0] += 1
                ps = psD[bank]
                for j in range(2):
                    first_pe = True
                    if si > 0:
                        # re-inject carried partial first
                        waits = []
                        if j == 0 and psD_hist[bank] is not None:
                            waits.extend(psD_hist[bank])
                        waits.append(evF[(li, lb, si - 1, j)])
                        P.emit("pe",
                               (lambda ps=ps, lb=lb, j=j: lambda e: e.matmul(
                                   ps[:, j * PB : (j + 1) * PB], lhsT=id_sb[:],
                                   rhs=pb_sb[:, (lb * 2 + j) * PB : (lb * 2 + j + 1) * PB],
                                   start=True, stop=False))(),
                               waits=fold(waits), inc="pe1")
                        first_pe = False
                        start = False
                    else:
                        start = True
                    smax = max(sbuild_t[(li, dm)] for dm in members)
                    for mi, dm in enumerate(members):
                        lr, lo, smr = chunk_loc[dm]
                        mw = []
                        if first_pe:
                            if j == 0 and psD_hist[bank] is not None:
                                mw.extend(psD_hist[bank])
                            first_pe = False
                        if gath_t[(li, dm)] is not None:
                            mw.append(gath_t[(li, dm)])
                        if mi == 0:
                            mw.append(("dve1", smax))
                        P.emit("pe",
                               (lambda ps=ps, lr=lr, lo=lo, j=j, smr=smr,
                                       st=(start and mi == 0),
                                       sp=(mi == len(members) - 1):
                                lambda e: e.matmul(
                                    ps[:, j * PB : (j + 1) * PB],
                                    lhsT=lr[:, lo + j * PB : lo + (j + 1) * PB],
                                    rhs=s_ring[:, smr * PB : (smr + 1) * PB],
                                    start=st, stop=sp))(),
                               waits=fold(mw), inc="pe1")
                        if j == 1:
                            mm_of_chunk[(li, dm)] = P.tick["pe1"]

                if last_of_sess:
                    mmt = P.tick["pe1"]
                    evac_ts = []
                    for j in range(2):
                        if last_sess:
                            t = P.emit("act",
                                       (lambda ps=ps, j=j, lb=lb, hout_key=hout_key,
                                               bgp_sb=bgp_sb: lambda e: e.activation(
                                           hT[hout_key][j][:, lb * PB : (lb + 1) * PB],
                                           ps[:, j * PB : (j + 1) * PB], AF.Relu,
                                           bias=bgp_sb[:, j : j + 1]))(),
                                       waits=[("pe1", mmt)], inc="act1")
                            evF[(li, lb)] = t
                            evF[(li, lb, si, j)] = ("act1", t)
                            evac_ts.append(("act1", t))
                            psD_hist[bank] = evac_ts
                            continue
                        else:
                            if PARTIAL_ACT:
                                t = P.emit("act",
                                           (lambda ps=ps, j=j, lb=lb: lambda e:
                                            e.activation(
                                                pb_sb[:, (lb * 2 + j) * PB : (lb * 2 + j + 1) * PB],
                                                ps[:, j * PB : (j + 1) * PB], AF.Copy))(),
                                           waits=[("pe1", mmt)], inc="act1")
                                evF[(li, lb, si, j)] = ("act1", t)
                                evac_ts.append(("act1", t))
                            else:
                                t = P.emit("dve",
                                           (lambda ps=ps, j=j, lb=lb: lambda e:
                                            e.tensor_copy(
                                                pb_sb[:, (lb * 2 + j) * PB : (lb * 2 + j + 1) * PB],
                                                ps[:, j * PB : (j + 1) * PB]))(),
                                           waits=[("pe1", mmt)], inc="dve1")
                                evF[(li, lb, si, j)] = ("dve1", t)
                                evac_ts.append(("dve1", t))
                            psD_hist[bank] = evac_ts
                            continue
                    psD_hist[bank] = evac_ts
                    ses_idx[lb] += 1
                    if last_sess:
                        after_final(lb)

        # ===================== schedule =====================
        # A chunks + B1 blocks, then sub-AG1s as slices complete
        done_ag1 = [False] * P_AG
        for si_ in range(NCH_A):
            emit_A_chunk(si_)
            for lb in range(4 * si_, min(NB, 4 * si_ + 4)):
                ready = ("act1", max(a_evt[(0, si_)], a_evt[(1, si_)]))
                emit_B_block(1, 1, wg1_sb, lb, ready)
                for g in range(P_AG):
                    if not done_ag1[g] and lb == cfg.BS[g + 1] - 1:
                        emit_subAG(1, g)
                        done_ag1[g] = True

        # layer 1 message passing; B2 + sub-AG2 follow block completion
        done_ag2 = [False] * P_AG
        b2_done = [False] * NB

        def after_l1(lb):
            ready = ("act1", evF[(1, lb)])
            emit_B_block(2, 2, wg2_sb, lb, ready)
            b2_done[lb] = True
            for g in range(P_AG):
                if not done_ag2[g] and all(
                        b2_done[l2] for l2 in range(cfg.BS[g], cfg.BS[g + 1])):
                    emit_subAG(2, g)
                    done_ag2[g] = True

        emit_layer(1, 2, bg1p_sb, after_l1)

        # ------------- stage G (h4 = relu(h3 @ W2 + b2)) + H (softmax out)
        evG = {}

        def emit_G_chunk(si):
            s = si * ACHUNK
            wd = min(ACHUNK, NLOC - s)
            bl0, bl1 = s // PB, (s + wd - 1) // PB
            ready = max(evF[(2, b_)] for b_ in range(bl0, bl1 + 1))
            for j in range(2):
                ai = len(psA_hist)
                ps = psA[ai % 2]
                for k in range(2):
                    waits = []
                    if k == 0:
                        waits = [("act1", ready), psA_hist[ai - 2]]
                    P.emit("pe",
                           (lambda ps=ps, j=j, s=s, wd=wd, k=k: lambda e: e.matmul(
                               ps[:, :wd],
                               lhsT=w2_sb[:, k * H + j * PB : k * H + (j + 1) * PB],
                               rhs=hT[3][k][:, s : s + wd],
                               start=(k == 0), stop=(k == 1)))(),
                           waits=fold(waits), inc="pe1")
                mmt = P.tick["pe1"]
                t = P.emit("act",
                           (lambda ps=ps, j=j, s=s, wd=wd: lambda e: e.activation(
                               hT[4][j][:, s : s + wd], ps[:, :wd], AF.Relu,
                               bias=b2p_sb[:, j : j + 1]))(),
                           waits=[("pe1", mmt)], inc="act1")
                evG[(j, si)] = t
                psA_hist.append(("act1", t))

        h_exp = {}
        h_mul = {}

        def emit_H_block(nb):
            bi = len(psB_hist)
            ps = psB[bi % 2]
            ready = ("act1", max(evG[(0, (nb * PB) // ACHUNK)],
                                 evG[(1, (nb * PB) // ACHUNK)]))
            for k in range(2):
                waits = []
                if k == 0:
                    waits = [ready] + list(psB_hist[bi - 2])
                P.emit("pe",
                       (lambda ps=ps, nb=nb, k=k: lambda e: e.matmul(
                           ps[:], lhsT=hT[4][k][:, nb * PB : (nb + 1) * PB],
                           rhs=w3_sb[:, k * DOUT : (k + 1) * DOUT],
                           start=(k == 0), stop=(k == 1)))(),
                       waits=fold(waits), inc="pe1")
            mmt = P.tick["pe1"]
            ls = (nb % 2) * DOUT
            addw = [("pe1", mmt)]
            if nb >= 2:
                addw.append(("act1", h_exp[nb - 2]))
            t_add = P.emit("dve",
                           (lambda ps=ps, ls=ls: lambda e: e.tensor_add(
                               lg_ring[:, ls : ls + DOUT], ps[:], b3bc_sb[:]))(),
                           waits=fold(addw), inc="dve1")
            psB_hist.append([("dve1", t_add)])
            cs = (nb % 4) * 2
            expw = [("dve1", t_add)]
            if nb >= 2:
                expw.append(("dve1", h_mul[nb - 2]))
            h_exp[nb] = P.emit(
                "act",
                (lambda ls=ls, cs=cs: lambda e: e.activation(
                    ex_ring[:, ls : ls + DOUT], lg_ring[:, ls : ls + DOUT], AF.Exp,
                    accum_out=sm_cols[:, cs : cs + 1]))(),
                waits=fold(expw), inc="act1")
            t_rec = P.emit("dve",
                           (lambda cs=cs: lambda e: e.reciprocal(
                               sm_cols[:, cs + 1 : cs + 2], sm_cols[:, cs : cs + 1]))(),
                           waits=[("act1", h_exp[nb])], inc="dve1")
            mulw = [("dve1", t_rec)]
            if nb >= 2:
                mulw.append(ot_hist[nb - 2])
            h_mul[nb] = P.emit(
                "dve",
                (lambda ls=ls, cs=cs: lambda e: e.tensor_scalar_mul(
                    ot_ring[:, ls : ls + DOUT], ex_ring[:, ls : ls + DOUT],
                    sm_cols[:, cs + 1 : cs + 2]))(),
                waits=fold(mulw), inc="dve1")
            oslot = nb % 2
            P.emit("sync",
                   (lambda nb=nb, ls=ls: lambda e: e.dma_start(
                       p_out[nb * PB : (nb + 1) * PB, :], ot_ring[:, ls : ls + DOUT]))(),
                   waits=[("dve1", h_mul[nb])], inc=f"ow{oslot}", inc_by=16)
            ot_hist.append((f"ow{oslot}", P.tick[f"ow{oslot}"]))

        # layer 2 message passing; G/H chunks follow block completion
        gh_done = set()

        def try_emit_gh(si):
            hi = min(NB, 4 * si + 4)
            if si in gh_done or not all(
                    (2, b_) in evF for b_ in range(4 * si, hi)):
                return
            gh_done.add(si)
            emit_G_chunk(si)
            for nb in range(4 * si, hi):
                emit_H_block(nb)

        emit_layer(2, 3, bg2p_sb, lambda lb: try_emit_gh(lb // 4))
        for si_ in range(NCH_A):
            try_emit_gh(si_)

        finw = [(s, P.tick[s]) for s in P.tick
                if s.startswith("fw") or s.startswith("ow")]
        P.emit("sync", lambda e: None, waits=finw)
        gfin = [(s, P.tick[s]) for s in P.tick if s.startswith("gs")]
        if gfin:
            P.emit("pool", lambda e: None, waits=gfin)

        # ------------- emit per-engine programs
        sems = {}
        semnames = ["c16", "cc1", "pe1", "dve1", "act1"]
        semnames += [f"gs{i}" for i in range(NRG)]
        semnames += [f"fw{i}" for i in range(4)]
        semnames += ["ow0", "ow1"]
        for s in semnames:
            sems[s] = ctx.enter_context(nc.semaphore(s))

        with nc.Block() as block:

            def mk_body(eng_name):
                def body(e):
                    last = {}
                    for fn, waits, inc, inc_by in P.ops[eng_name]:
                        for s, v in waits:
                            if v is None or last.get(s, 0) >= v:
                                continue
                            e.wait_ge(sems[s], v)
                            last[s] = v
                        ins = fn(e)
                        if inc is not None and ins is not None:
                            ins.then_inc(sems[inc], inc_by)
                return body

            block.sync(mk_body("sync"))
            block.tensor(mk_body("pe"))
            block.vector(mk_body("dve"))
            block.scalar(mk_body("act"))
            block.gpsimd(mk_body("pool"))

    if GQN > 1:
        # round-robin indirect gathers across the SWDGE queues
        k = 0
        for blk in nc.m.functions[0].blocks:
            for ins_ in getattr(blk, "instructions", []):
                if (type(ins_).__name__ == "InstDMACopy"
                        and getattr(ins_, "queue", None) == "qPoolDynamic"
                        and any(getattr(a, "dynamic_ap_info", None) is not None
                                for a in ins_.ins
                                if hasattr(a, "dynamic_ap_info"))):
                    q = k % GQN
                    if q:
                        ins_.queue = f"qPoolDynamic{q}"
                    k += 1

    return nc


# ---------------------------------------------------------------- entry point
def build_in_maps(cfg, prep, wts):
    in_maps = []
    for c in range(C):
        in_maps.append(dict(
            xT=prep["xT"][c], W1=wts["W1"], b1p=wts["b1p"], wg1p=wts["wg1p"],
            bg1p=wts["bg1p"], wg2p=wts["wg2p"], bg2p=wts["bg2p"], w2p=wts["w2p"],
            b2p=wts["b2p"], w3p=wts["w3p"], b3bc=wts["b3bc"], iota=wts["iota"],
            ident=wts["ident"],
            esrcT=prep["esrcT"][c], edstT=prep["edstT"][c],
            enormT=prep["enormT"][c], dinv_col=prep["dinv_col"][c],
        ))
    return in_maps


def run(cfg, inputs, trace=False):
    prep = preprocess(cfg, inputs["x"], inputs["edge_index"])
    wts = pack_weights(
        cfg,
        inputs["W1"], inputs["b1"], inputs["Wg1"], inputs["bg1"],
        inputs["Wg2"], inputs["bg2"], inputs["W2"], inputs["b2"],
        inputs["W3"], inputs["b3"],
    )
    nc = build_graph(cfg, prep)
    in_maps = build_in_maps(cfg, prep, wts)
    res = run_bass_kernel_spmd(nc, in_maps, list(range(C)), trace=trace)
    shards = np.concatenate([np.asarray(res.results[c]["out"]) for c in range(C)], axis=0)
    out = shards[prep["new_row"]]
    return np.ascontiguousarray(out, dtype=np.float32), res


def kernel(**inputs):
    out, _ = run(FULL, inputs, trace=False)
    return out
